# revision 51
# baseline (speedup 1.0000x reference)
"""Pointer-style attention kernel for Trainium2, SPMD over 8 NeuronCores.

Reference computation (per full batch B=128, S=2048, E=H=512):
    q  = query @ Wq.T + bq                    [B, H]
    k  = target @ Wk.T + bk                   [B, S, H]
    qk = einsum('bh,bsh->bs', q, k)           [B, S]
    qk = 10 * tanh(qk);  qk[mask==1] = -inf
    alpha = softmax(qk, axis=-1)

Key algebraic reformulation (exact in exact arithmetic):
    qk[b,s] = target[b,s,:] . qp[b,:] + qb[b]
      qp = (query @ Wq.T + bq) @ Wk           [B, E]
      qb = (query @ Wq.T + bq) . bk           [B]
This collapses the S*E*H einsum (137 GFLOP) into an S*E dot-product
stream (0.27 GFLOP), making the kernel HBM-bound on streaming `target`
(64 MiB per core; ~187 us floor at the 358 GB/s per-core HBM limit).
qp/qb are tiny (B x E) and are precomputed on the host alongside the
other layout prep, so the device spends no stream time on weights.

Distribution: data-parallel over batch; 16 batches per core, weights
replicated, no cross-core communication (softmax is per-row).

Per-core plan (the DVE is the critical path; everything else is shaped
to never make it wait):
  - target streams as 32 half-batch units of [128, 8, 512] fp32,
    alternating the two HWDGE rings (sync/SP and scalar/ACT) so per-DMA
    completion latency on one ring hides under the other's stream; unit
    DMAs are emitted 5 units ahead of their consuming compute so a
    dispatch never waits behind anything on its sequencer. The
    s<->(partition,row) mapping s = 1024h + 8p + j makes each
    partition's 16 KB contiguous in HBM (fat descriptors); the
    resulting output permutation is undone on the host. First/last
    units are quarter-split so compute starts after 512 KB and the
    final reduce trails the stream end by ~1.4 us.
  - one fused DVE scalar_tensor_tensor per (batch, s-row) does
    mul+reduce in a single pass: scores[:,b,c] = sum_e target*qp, with
    the product sunk into a stride-0 dummy. in1 reads from SBUF (a
    PSUM in1 costs +85 ns per call), so all 16 qp partition-broadcasts
    (TensorE matmuls vs identity) are staged through SBUF in the
    preamble.
  - epilogue per 2 batches, deferred 2 units past the pair's last unit
    (emitted at the boundary, its cross-engine chain head-of-line
    blocks the in-order queues): tanh/exp on ScalarE with qb folded
    into the tanh bias, fused mask+row-sum and normalize on DVE
    (~0.7 us/pair), denominator sums and output transpose on TensorE.
    Outputs collect in SBUF and leave as one DMA at the end — mid-
    stream out-DMAs block later target units on their HWDGE ring, and
    any GpSimd/SWDGE activity throttles the SDMA engines (412 -> 334
    GB/s measured).
"""

import sys
import types

import numpy as np

B, S, E, H = 128, 2048, 512, 512
C_CLIP = 10.0
NCORES = 8
BS = B // NCORES  # 16 batches per core
HK = 8  # s-rows per partition per unit; s = 1024h + 8p + j
CPB = 16  # score columns per batch (c = 8h + j)
NU = BS * 2  # 32 half-batch pipeline units


def _install_axon_profile_shim():
    """Make run_bass_kernel_spmd(trace=True) usable in this container:
    provide antenv.axon_hooks (NTFF profile hook via ctypes into the
    axon PJRT .so) and stub the S3 artifact upload."""
    try:
        if "antenv.axon_hooks" not in sys.modules:
            import antenv
            from trn_agent_boot.trn_boot import _ntff_profile_via_ctypes

            hook = _ntff_profile_via_ctypes("/opt/axon/libaxon_pjrt.so")
            mod = types.ModuleType("antenv.axon_hooks")
            mod._hook = hook
            mod.get_axon_ntff_profile_hook = lambda: mod._hook

            def _set(h):
                mod._hook = h

            mod.set_axon_ntff_profile_hook = _set
            sys.modules["antenv.axon_hooks"] = mod
            antenv.axon_hooks = mod
    except Exception:
        pass
    try:
        import concourse.bass_utils as bu

        bu.upload_artifacts = lambda tmpdir: str(tmpdir)
    except Exception:
        pass


def _legalize_sync_waits(nc):
    """This walrus build rejects instructions carrying more than a couple
    of sync-wait commands. After Tile scheduling, split each instruction's
    excess waits onto same-engine NOPs inserted immediately before it —
    sequencers execute in order, so semantics are identical."""
    import bass_rust
    from concourse import mybir

    n_split = 0
    for f in nc.m.functions:
        for blk in f.blocks:
            il = blk.instructions
            out = []
            changed = False
            for inst in il:
                si = inst.sync_info
                waits = list(si.on_wait) if si is not None else []
                cap = 2 if isinstance(inst, mybir.InstEventSemaphore) else 1
                if len(waits) > cap:
                    rest = waits[: len(waits) - cap]
                    for j, w in enumerate(rest):
                        nop = mybir.InstNoOp(
                            name=f"{inst.name}-swait{j}",
                            engine=inst.engine,
                            bass_nofuse=True,
                            sync_info=bass_rust.SyncInfo(on_wait=[w], on_update=[]),
                        )
                        out.append(nop)
                        n_split += 1
                    si.on_wait = waits[len(waits) - cap :]
                    inst.sync_info = si
                    changed = True
                out.append(inst)
            if changed:
                blk.instructions = out
    return n_split


def build_kernel():
    import concourse.bass as bass
    import concourse.tile as tile
    from concourse import mybir
    from concourse.masks import make_identity

    f32 = mybir.dt.float32
    bf16 = mybir.dt.bfloat16
    Alu = mybir.AluOpType
    Act = mybir.ActivationFunctionType

    nc = bass.Bass()
    # host passes qp/qb precomputed and mask as a permuted keep-multiplier
    target_d = nc.dram_tensor("target", [BS, S, E], f32, kind="ExternalInput")
    qpT_d = nc.dram_tensor("qpT", [128, 4 * BS], f32, kind="ExternalInput")
    pbb2_d = nc.dram_tensor("pbb2", [128, 2 * E], f32, kind="ExternalInput")
    qbb_d = nc.dram_tensor("qbb", [128, BS], f32, kind="ExternalInput")
    m01P_d = nc.dram_tensor("m01P", [128, BS * CPB], f32, kind="ExternalInput")
    alphaP_d = nc.dram_tensor("alphaP", [BS * CPB, 128], f32, kind="ExternalOutput")

    # unit (b, h): partition p holds s-rows 1024h + 8p + j, j=0..7 —
    # 16 KB contiguous per partition per unit
    units = target_d.rearrange("b (h p k) e -> (b h) p k e", h=2, p=128, k=HK)

    with tile.TileContext(nc) as tc:
        with (
            tc.tile_pool(name="singles", bufs=1) as singles,
            tc.tile_pool(name="tgt", bufs=10) as tgtp,
            tc.tile_pool(name="epi", bufs=2) as epip,
            tc.tile_pool(name="ppre", bufs=2, space="PSUM") as ppre,
            tc.tile_pool(name="pqpb", bufs=2, space="PSUM") as pqpb,
            tc.tile_pool(name="pepi", bufs=2, space="PSUM") as pepi,
        ):
            # small inputs: qpT/qbb at the head of the sync ring, m01P on
            # the scalar ring; target units alternate both rings behind.
            # qpT[p, c, b] = qp[b, 128c+p]: each batch's qp column chunks sit
            # on the partition axis, ready for a stride-0 broadcast matmul.
            qpT_sb = singles.tile([128, 4, BS], f32)
            nc.sync.dma_start(
                out=qpT_sb, in_=qpT_d.rearrange("p (c b) -> p c b", b=BS)
            )
            # batches 0-1 arrive pre-broadcast from the host (4 KB/part) so
            # the first STT is gated only by the first target quarter-unit,
            # not the qpT -> matmul -> copy chain (~3 us earlier start)
            pbs = singles.tile([128, BS, E], f32)
            nc.sync.dma_start(
                out=pbs[:, 0:2, :],
                in_=pbb2_d.rearrange("p (b e) -> p b e", b=2),
            )
            qbb = singles.tile([128, BS], f32)
            nc.sync.dma_start(out=qbb, in_=qbb_d[:, :])
            m01T = singles.tile([128, BS, CPB], f32)  # keep-multiplier
            nc.scalar.dma_start(
                out=m01T, in_=m01P_d.rearrange("p (b c) -> p b c", b=BS)
            )

            ident = singles.tile([128, 128], f32)
            make_identity(nc, ident)
            ones_row = singles.tile([1, 128], f32)  # lhsT for partition-bcast
            nc.vector.memset(ones_row, 1.0)
            ones_col = singles.tile([128, 1], f32)  # lhsT for partition-sum
            nc.vector.memset(ones_col, 1.0)

            # Two PE warmup matmuls to start the HAM clock ramp while the
            # first target unit lands.
            for _ in range(2):
                pwrm = ppre.tile([128, 128], f32, tag="pre")
                nc.tensor.matmul(pwrm, ident, ident, start=True, stop=True)

            scores = singles.tile([128, BS, CPB], f32)
            e2 = singles.tile([128, BS, CPB], f32)
            a_sb = singles.tile([128, BS, CPB], f32)
            part = singles.tile([128, BS], f32)
            dummy = singles.tile([128, 1], f32)  # stride-0 sink for STT out
            # transposed output staging: all 8 pairs land here, one DMA at
            # the end (mid-stream out-DMAs would either block later target
            # units on an HWDGE ring or run on the slower SWDGE path)
            atall = singles.tile([32, 8, 128], f32)

            def _epi_pair(b0):
                """tanh/exp/mask/normalize batches b0, b0+1 into atall.
                ScalarE does the activations (qb folds into the tanh bias),
                TensorE the partition sums and transpose; the small fused
                mask+row-sum and normalize stay on the DVE — on Scalar they
                head-of-line block its queue, on GpSimd they throttle the
                SDMA engines."""
                for b in (b0, b0 + 1):
                    t_t = epip.tile([128, CPB], f32, tag="tanh")
                    nc.scalar.activation(
                        t_t, scores[:, b, :], Act.Tanh,
                        bias=qbb[:, b : b + 1], scale=1.0,
                    )
                    nc.scalar.activation(e2[:, b, :], t_t, Act.Exp, scale=C_CLIP)
                    # fused: e2 *= m01 (mask) and part = row-sum, one pass
                    # (STT is DVE-only; at [128,16] it costs ~90 ns)
                    nc.vector.scalar_tensor_tensor(
                        out=e2[:, b, :], in0=e2[:, b, :], scalar=0.0,
                        in1=m01T[:, b, :], op0=Alu.bypass, op1=Alu.mult,
                        accum_out=part[:, b : b + 1],
                    )
                pden = pepi.tile([1, 2], f32, tag="epi")
                nc.tensor.matmul(
                    pden, ones_col, part[:, b0 : b0 + 2], start=True, stop=True
                )
                recip = epip.tile([1, 2], f32, tag="recip")
                nc.vector.reciprocal(recip, pden)
                prb = pepi.tile([128, 2], f32, tag="epi")
                nc.tensor.matmul(prb, ones_row, recip, start=True, stop=True)
                rb = epip.tile([128, 2], f32, tag="rb")
                nc.vector.tensor_copy(rb, prb)
                for b in (b0, b0 + 1):
                    # normalize on DVE: placing any V-dependent op on
                    # Scalar's in-order queue head-of-line-blocks its DMA
                    # dispatches (measured +24 us)
                    nc.vector.tensor_scalar(
                        out=a_sb[:, b, :], in0=e2[:, b, :],
                        scalar1=rb[:, b - b0 : b - b0 + 1], scalar2=None,
                        op0=Alu.mult,
                    )
                pat = pepi.tile([32, 128], f32, tag="epi")
                nc.tensor.transpose(pat, a_sb[:, b0 : b0 + 2, :], ident)
                nc.scalar.copy(atall[:, b0 // 2, :], pat)

            # ---- main pipeline: stream target; one fused mul+reduce per
            # s-row on DVE (STT from SBUF runs at plain-multiply speed;
            # reading in1 from PSUM costs +85 ns/call, so qp broadcasts are
            # staged through SBUF by ScalarE). Target units alternate the
            # two HWDGE rings (SP/sync and ACT/scalar); their dispatches are
            # emitted LOOK units ahead so a dispatch never waits behind
            # anything on the issuing sequencer.
            LOOK = 5

            tgt_tiles = {}

            def emit_dma(u):
                tgt = tgtp.tile([128, HK, E], f32, tag="tgt")
                tgt_tiles[u] = tgt
                eng = nc.sync if (u % 2 == 0) else nc.scalar
                if u < 4 or u >= NU - 2:
                    # quarter-split the first unit on each ring (compute can
                    # start after the first 512 KB lands) and the last two
                    # (the final reduce trails the stream end by ~1.4 us)
                    for j0 in range(0, HK, 2):
                        eng.dma_start(
                            out=tgt[:, j0 : j0 + 2, :],
                            in_=units[u][:, j0 : j0 + 2, :],
                        )
                else:
                    eng.dma_start(out=tgt, in_=units[u])

            for u in range(LOOK):
                emit_dma(u)

            # all 16 qp partition-broadcasts up front: TensorE matmuls into
            # PSUM, ScalarE copies to SBUF (STT's in1 from PSUM costs +85
            # ns/call; staged in SBUF it runs at plain-multiply speed).
            # Doing them all in the preamble keeps ScalarE's in-order queue
            # empty mid-stream — anything queued there blocks the pair
            # epilogues' tanh and the ACT-ring DMA dispatches behind it.
            for b in range(2, BS):
                pbp = pqpb.tile([128, E], f32, tag="qpb")
                for c in range(4):
                    qrep = bass.AP(
                        tensor=qpT_sb.tensor,
                        offset=qpT_sb[:, c, b : b + 1].offset,
                        ap=[qpT_sb.ap[0], [0, 128]],
                    )
                    nc.tensor.matmul(
                        pbp[:, c * 128 : (c + 1) * 128], qrep, ident,
                        start=True, stop=True,
                    )
                nc.scalar.copy(pbs[:, b, :], pbp)

            for u in range(NU):
                if u + LOOK < NU:
                    emit_dma(u + LOOK)
                b, h = divmod(u, 2)
                tgt = tgt_tiles.pop(u)
                for j in range(HK):
                    nc.vector.scalar_tensor_tensor(
                        out=dummy.broadcast_to((128, E)),
                        in0=tgt[:, j, :],
                        scalar=0.0,
                        in1=pbs[:, b, :],
                        op0=Alu.bypass,
                        op1=Alu.mult,
                        accum_out=scores[:, b, h * HK + j : h * HK + j + 1],
                    )
                # pair epilogue deferred 2 units: when emitted right at the
                # pair's last unit, V's mask-STT waits on Scalar's exp which
                # waits on V's own just-emitted scores — a ~0.6 us V stall
                # per pair
                if u >= 5 and (u - 5) % 4 == 0:
                    _epi_pair((u - 5) // 2)
            _epi_pair(BS - 2)

            # single output DMA: alphaP rows are pair-major (b*CPB + c)
            nc.sync.dma_start(
                out=alphaP_d.rearrange("(k r) p -> r k p", k=8), in_=atall
            )

    _legalize_sync_waits(nc)
    return nc


_NC_CACHE = None


def kernel(query, target, mask, Wq, bq, Wk, bk):
    global _NC_CACHE
    _install_axon_profile_shim()
    from concourse.bass_utils import run_bass_kernel_spmd

    query = np.ascontiguousarray(np.asarray(query, dtype=np.float32))
    target = np.ascontiguousarray(np.asarray(target, dtype=np.float32))
    mask = np.ascontiguousarray(np.asarray(mask, dtype=np.int32))
    Wq = np.ascontiguousarray(np.asarray(Wq, dtype=np.float32))
    bq = np.ascontiguousarray(np.asarray(bq, dtype=np.float32))
    Wk = np.ascontiguousarray(np.asarray(Wk, dtype=np.float32))
    bk = np.ascontiguousarray(np.asarray(bk, dtype=np.float32))

    if _NC_CACHE is None:
        _NC_CACHE = build_kernel()
    nc = _NC_CACHE

    in_maps = make_in_maps(query, target, mask, Wq, bq, Wk, bk)

    res = run_bass_kernel_spmd(nc, in_maps, list(range(NCORES)))
    outs = []
    for i in range(NCORES):
        aP = np.asarray(res.results[i]["alphaP"])  # [BS*CPB, 128]
        # undo the s = 1024h + 8p + j permutation
        a = aP.reshape(BS, 2, HK, 128).transpose(0, 1, 3, 2).reshape(BS, S)
        outs.append(a)
    return np.concatenate(outs, axis=0).astype(np.float32)


def make_in_maps(query, target, mask, Wq, bq, Wk, bk):
    # tiny derived tensors (B x E): q = query @ Wq.T + bq, qp = q @ Wk,
    # qb = q . bk — O(B*E*H) host prep vs the O(B*S*E) device stream
    q = query @ Wq.T + bq  # [B, H]
    qp_full = (q @ Wk).astype(np.float32)  # [B, E]
    qb_full = (q @ bk).astype(np.float32)  # [B]
    in_maps = []
    for i in range(NCORES):
        sl = slice(i * BS, (i + 1) * BS)
        m01 = (mask[sl] == 0).astype(np.float32)  # 1.0 keep / 0.0 masked
        m01P = np.ascontiguousarray(
            m01.reshape(BS, 2, 128, HK).transpose(2, 0, 1, 3).reshape(128, BS * CPB)
        )
        qbb = np.ascontiguousarray(
            np.broadcast_to(qb_full[sl][None, :], (128, BS)).astype(np.float32)
        )
        in_maps.append(
            {
                "target": target[sl],
                "pbb2": np.ascontiguousarray(
                    np.broadcast_to(
                        qp_full[sl][0:2].reshape(1, 2 * E), (128, 2 * E)
                    )
                ),
                "qpT": np.ascontiguousarray(
                    qp_full[sl].reshape(BS, 4, 128).transpose(2, 1, 0).reshape(128, 4 * BS)
                ),
                "qbb": qbb,
                "m01P": m01P,
            }
        )
    return in_maps


# revision 53
# speedup vs baseline: 1.0207x; 1.0207x over previous
"""Pointer-style attention kernel for Trainium2, SPMD over 8 NeuronCores.

Reference computation (per full batch B=128, S=2048, E=H=512):
    q  = query @ Wq.T + bq                    [B, H]
    k  = target @ Wk.T + bk                   [B, S, H]
    qk = einsum('bh,bsh->bs', q, k)           [B, S]
    qk = 10 * tanh(qk);  qk[mask==1] = -inf
    alpha = softmax(qk, axis=-1)

Key algebraic reformulation (exact in exact arithmetic):
    qk[b,s] = target[b,s,:] . qp[b,:] + qb[b]
      qp = (query @ Wq.T + bq) @ Wk           [B, E]
      qb = (query @ Wq.T + bq) . bk           [B]
This collapses the S*E*H einsum (137 GFLOP) into an S*E dot-product
stream (0.27 GFLOP), making the kernel HBM-bound on streaming `target`
(64 MiB per core; ~187 us floor at the 358 GB/s per-core HBM limit).
qp/qb are tiny (B x E) and are precomputed on the host alongside the
other layout prep, so the device spends no stream time on weights.

Distribution: data-parallel over batch; 16 batches per core, weights
replicated, no cross-core communication (softmax is per-row).

Per-core plan (the DVE is the critical path; everything else is shaped
to never make it wait):
  - target streams as 32 half-batch units of [128, 8, 512] fp32,
    alternating the two HWDGE rings (sync/SP and scalar/ACT) so per-DMA
    completion latency on one ring hides under the other's stream; unit
    DMAs are emitted 5 units ahead of their consuming compute so a
    dispatch never waits behind anything on its sequencer. The
    s<->(partition,row) mapping s = 1024h + 8p + j makes each
    partition's 16 KB contiguous in HBM (fat descriptors); the
    resulting output permutation is undone on the host. First/last
    units are quarter-split so compute starts after 512 KB and the
    final reduce trails the stream end by ~1.4 us.
  - one fused DVE scalar_tensor_tensor per (batch, s-row) does
    mul+reduce in a single pass: scores[:,b,c] = sum_e target*qp, with
    the product sunk into a stride-0 dummy. in1 reads from SBUF (a
    PSUM in1 costs +85 ns per call), so all 16 qp partition-broadcasts
    (TensorE matmuls vs identity) are staged through SBUF in the
    preamble.
  - epilogue per 2 batches, deferred 2 units past the pair's last unit
    (emitted at the boundary, its cross-engine chain head-of-line
    blocks the in-order queues): tanh/exp on ScalarE with qb folded
    into the tanh bias, fused mask+row-sum and normalize on DVE
    (~0.7 us/pair), denominator sums and output transpose on TensorE.
    Outputs collect in SBUF and leave as one DMA at the end — mid-
    stream out-DMAs block later target units on their HWDGE ring, and
    any GpSimd/SWDGE activity throttles the SDMA engines (412 -> 334
    GB/s measured).
"""

import sys
import types

import numpy as np

B, S, E, H = 128, 2048, 512, 512
C_CLIP = 10.0
NCORES = 8
BS = B // NCORES  # 16 batches per core
HK = 8  # s-rows per partition per unit; s = 1024h + 8p + j
CPB = 16  # score columns per batch (c = 8h + j)
NU = BS * 2  # 32 half-batch pipeline units


def _install_axon_profile_shim():
    """Make run_bass_kernel_spmd(trace=True) usable in this container:
    provide antenv.axon_hooks (NTFF profile hook via ctypes into the
    axon PJRT .so) and stub the S3 artifact upload."""
    try:
        if "antenv.axon_hooks" not in sys.modules:
            import antenv
            from trn_agent_boot.trn_boot import _ntff_profile_via_ctypes

            hook = _ntff_profile_via_ctypes("/opt/axon/libaxon_pjrt.so")
            mod = types.ModuleType("antenv.axon_hooks")
            mod._hook = hook
            mod.get_axon_ntff_profile_hook = lambda: mod._hook

            def _set(h):
                mod._hook = h

            mod.set_axon_ntff_profile_hook = _set
            sys.modules["antenv.axon_hooks"] = mod
            antenv.axon_hooks = mod
    except Exception:
        pass
    try:
        import concourse.bass_utils as bu

        bu.upload_artifacts = lambda tmpdir: str(tmpdir)
    except Exception:
        pass


def _legalize_sync_waits(nc):
    """This walrus build rejects instructions carrying more than a couple
    of sync-wait commands. After Tile scheduling, split each instruction's
    excess waits onto same-engine NOPs inserted immediately before it —
    sequencers execute in order, so semantics are identical."""
    import bass_rust
    from concourse import mybir

    n_split = 0
    for f in nc.m.functions:
        for blk in f.blocks:
            il = blk.instructions
            out = []
            changed = False
            for inst in il:
                si = inst.sync_info
                waits = list(si.on_wait) if si is not None else []
                cap = 2 if isinstance(inst, mybir.InstEventSemaphore) else 1
                if len(waits) > cap:
                    rest = waits[: len(waits) - cap]
                    for j, w in enumerate(rest):
                        nop = mybir.InstNoOp(
                            name=f"{inst.name}-swait{j}",
                            engine=inst.engine,
                            bass_nofuse=True,
                            sync_info=bass_rust.SyncInfo(on_wait=[w], on_update=[]),
                        )
                        out.append(nop)
                        n_split += 1
                    si.on_wait = waits[len(waits) - cap :]
                    inst.sync_info = si
                    changed = True
                out.append(inst)
            if changed:
                blk.instructions = out
    return n_split


def build_kernel():
    import concourse.bass as bass
    import concourse.tile as tile
    from concourse import mybir
    from concourse.masks import make_identity

    f32 = mybir.dt.float32
    bf16 = mybir.dt.bfloat16
    Alu = mybir.AluOpType
    Act = mybir.ActivationFunctionType

    nc = bass.Bass()
    # host passes qp/qb precomputed and mask as a permuted keep-multiplier
    target_d = nc.dram_tensor("target", [BS, S, E], f32, kind="ExternalInput")
    qpT_d = nc.dram_tensor("qpT", [128, 4 * BS], f32, kind="ExternalInput")
    pbb2_d = nc.dram_tensor("pbb2", [128, 2 * E], f32, kind="ExternalInput")
    qbb_d = nc.dram_tensor("qbb", [128, BS], f32, kind="ExternalInput")
    m01P_d = nc.dram_tensor("m01P", [128, BS * CPB], f32, kind="ExternalInput")
    alphaP_d = nc.dram_tensor("alphaP", [BS * CPB, 128], f32, kind="ExternalOutput")

    # unit (b, h): partition p holds s-rows 1024h + 8p + j, j=0..7 —
    # 16 KB contiguous per partition per unit
    units = target_d.rearrange("b (h p k) e -> (b h) p k e", h=2, p=128, k=HK)

    with tile.TileContext(nc) as tc:
        with (
            tc.tile_pool(name="singles", bufs=1) as singles,
            tc.tile_pool(name="tgt", bufs=10) as tgtp,
            tc.tile_pool(name="epi", bufs=2) as epip,
            tc.tile_pool(name="ppre", bufs=2, space="PSUM") as ppre,
            tc.tile_pool(name="pqpb", bufs=2, space="PSUM") as pqpb,
            tc.tile_pool(name="pepi", bufs=2, space="PSUM") as pepi,
        ):
            # small inputs: qpT/qbb at the head of the sync ring, m01P on
            # the scalar ring; target units alternate both rings behind.
            # qpT[p, c, b] = qp[b, 128c+p]: each batch's qp column chunks sit
            # on the partition axis, ready for a stride-0 broadcast matmul.
            qpT_sb = singles.tile([128, 4, BS], f32)
            nc.sync.dma_start(
                out=qpT_sb, in_=qpT_d.rearrange("p (c b) -> p c b", b=BS)
            )
            # batches 0-1 arrive pre-broadcast from the host (4 KB/part) so
            # the first STT is gated only by the first target quarter-unit,
            # not the qpT -> matmul -> copy chain (~3 us earlier start)
            pbs = singles.tile([128, BS, E], f32)
            nc.sync.dma_start(
                out=pbs[:, 0:2, :],
                in_=pbb2_d.rearrange("p (b e) -> p b e", b=2),
            )
            qbb = singles.tile([128, BS], f32)
            nc.sync.dma_start(out=qbb, in_=qbb_d[:, :])
            m01T = singles.tile([128, BS, CPB], f32)  # keep-multiplier
            nc.scalar.dma_start(
                out=m01T, in_=m01P_d.rearrange("p (b c) -> p b c", b=BS)
            )

            ident = singles.tile([128, 128], f32)
            make_identity(nc, ident)
            ones_row = singles.tile([1, 128], f32)  # lhsT for partition-bcast
            nc.vector.memset(ones_row, 1.0)
            ones_col = singles.tile([128, 1], f32)  # lhsT for partition-sum
            nc.vector.memset(ones_col, 1.0)

            # Two PE warmup matmuls to start the HAM clock ramp while the
            # first target unit lands.
            for _ in range(2):
                pwrm = ppre.tile([128, 128], f32, tag="pre")
                nc.tensor.matmul(pwrm, ident, ident, start=True, stop=True)

            scores = singles.tile([128, BS, CPB], f32)
            e2 = singles.tile([128, BS, CPB], f32)
            a_sb = singles.tile([128, BS, CPB], f32)
            part = singles.tile([128, BS], f32)
            dummy = singles.tile([128, 1], f32)  # stride-0 sink for STT out
            # transposed output staging: all 8 pairs land here, one DMA at
            # the end (mid-stream out-DMAs would either block later target
            # units on an HWDGE ring or run on the slower SWDGE path)
            atall = singles.tile([32, 8, 128], f32)

            def _epi_pair(b0):
                """tanh/exp/mask/normalize batches b0, b0+1 into atall.
                ScalarE does the activations (qb folds into the tanh bias),
                TensorE the partition sums and transpose; the small fused
                mask+row-sum and normalize stay on the DVE — on Scalar they
                head-of-line block its queue, on GpSimd they throttle the
                SDMA engines."""
                for b in (b0, b0 + 1):
                    t_t = epip.tile([128, CPB], f32, tag="tanh")
                    nc.scalar.activation(
                        t_t, scores[:, b, :], Act.Tanh,
                        bias=qbb[:, b : b + 1], scale=1.0,
                    )
                    nc.scalar.activation(e2[:, b, :], t_t, Act.Exp, scale=C_CLIP)
                    # fused: e2 *= m01 (mask) and part = row-sum, one pass
                    # (STT is DVE-only; at [128,16] it costs ~90 ns)
                    nc.vector.scalar_tensor_tensor(
                        out=e2[:, b, :], in0=e2[:, b, :], scalar=0.0,
                        in1=m01T[:, b, :], op0=Alu.bypass, op1=Alu.mult,
                        accum_out=part[:, b : b + 1],
                    )
                pden = pepi.tile([1, 2], f32, tag="epi")
                nc.tensor.matmul(
                    pden, ones_col, part[:, b0 : b0 + 2], start=True, stop=True
                )
                recip = epip.tile([1, 2], f32, tag="recip")
                nc.vector.reciprocal(recip, pden)
                prb = pepi.tile([128, 2], f32, tag="epi")
                nc.tensor.matmul(prb, ones_row, recip, start=True, stop=True)
                rb = epip.tile([128, 2], f32, tag="rb")
                nc.vector.tensor_copy(rb, prb)
                for b in (b0, b0 + 1):
                    # normalize on DVE: placing any V-dependent op on
                    # Scalar's in-order queue head-of-line-blocks its DMA
                    # dispatches (measured +24 us)
                    nc.vector.tensor_scalar(
                        out=a_sb[:, b, :], in0=e2[:, b, :],
                        scalar1=rb[:, b - b0 : b - b0 + 1], scalar2=None,
                        op0=Alu.mult,
                    )
                pat = pepi.tile([32, 128], f32, tag="epi")
                nc.tensor.transpose(pat, a_sb[:, b0 : b0 + 2, :], ident)
                nc.scalar.copy(atall[:, b0 // 2, :], pat)

            # ---- main pipeline: stream target; one fused mul+reduce per
            # s-row on DVE (STT from SBUF runs at plain-multiply speed;
            # reading in1 from PSUM costs +85 ns/call, so qp broadcasts are
            # staged through SBUF by ScalarE). Target units alternate the
            # two HWDGE rings (SP/sync and ACT/scalar); their dispatches are
            # emitted LOOK units ahead so a dispatch never waits behind
            # anything on the issuing sequencer.
            LOOK = 5

            tgt_tiles = {}

            def emit_dma(u):
                tgt = tgtp.tile([128, HK, E], f32, tag="tgt")
                tgt_tiles[u] = tgt
                eng = nc.sync if (u % 2 == 0) else nc.scalar
                if u < 4 or u >= NU - 2:
                    # quarter-split the first unit on each ring (compute can
                    # start after the first 512 KB lands) and the last two
                    # (the final reduce trails the stream end by ~1.4 us)
                    for j0 in range(0, HK, 2):
                        eng.dma_start(
                            out=tgt[:, j0 : j0 + 2, :],
                            in_=units[u][:, j0 : j0 + 2, :],
                        )
                else:
                    eng.dma_start(out=tgt, in_=units[u])

            for u in range(LOOK):
                emit_dma(u)

            # all 16 qp partition-broadcasts up front: TensorE matmuls into
            # PSUM, ScalarE copies to SBUF (STT's in1 from PSUM costs +85
            # ns/call; staged in SBUF it runs at plain-multiply speed).
            # Doing them all in the preamble keeps ScalarE's in-order queue
            # empty mid-stream — anything queued there blocks the pair
            # epilogues' tanh and the ACT-ring DMA dispatches behind it.
            for b in range(2, BS):
                pbp = pqpb.tile([128, E], f32, tag="qpb")
                for c in range(4):
                    qrep = bass.AP(
                        tensor=qpT_sb.tensor,
                        offset=qpT_sb[:, c, b : b + 1].offset,
                        ap=[qpT_sb.ap[0], [0, 128]],
                    )
                    nc.tensor.matmul(
                        pbp[:, c * 128 : (c + 1) * 128], qrep, ident,
                        start=True, stop=True,
                    )
                nc.scalar.copy(pbs[:, b, :], pbp)

            for u in range(NU):
                if u + LOOK < NU:
                    emit_dma(u + LOOK)
                b, h = divmod(u, 2)
                tgt = tgt_tiles.pop(u)
                for j in range(HK):
                    nc.vector.scalar_tensor_tensor(
                        out=dummy.broadcast_to((128, E)),
                        in0=tgt[:, j, :],
                        scalar=0.0,
                        in1=pbs[:, b, :],
                        op0=Alu.bypass,
                        op1=Alu.mult,
                        accum_out=scores[:, b, h * HK + j : h * HK + j + 1],
                    )
                # pair epilogue deferred 2 units: when emitted right at the
                # pair's last unit, V's mask-STT waits on Scalar's exp which
                # waits on V's own just-emitted scores — a ~0.6 us V stall
                # per pair
                if u >= 5 and (u - 5) % 4 == 0:
                    _epi_pair((u - 5) // 2)
            _epi_pair(BS - 2)

            # single output DMA: alphaP rows are pair-major (b*CPB + c)
            nc.sync.dma_start(
                out=alphaP_d.rearrange("(k r) p -> r k p", k=8), in_=atall
            )

    _legalize_sync_waits(nc)
    return nc


_NC_CACHE = None


def kernel(query, target, mask, Wq, bq, Wk, bk):
    global _NC_CACHE
    _install_axon_profile_shim()
    from concourse.bass_utils import run_bass_kernel_spmd

    query = np.ascontiguousarray(np.asarray(query, dtype=np.float32))
    target = np.ascontiguousarray(np.asarray(target, dtype=np.float32))
    mask = np.ascontiguousarray(np.asarray(mask, dtype=np.int32))
    Wq = np.ascontiguousarray(np.asarray(Wq, dtype=np.float32))
    bq = np.ascontiguousarray(np.asarray(bq, dtype=np.float32))
    Wk = np.ascontiguousarray(np.asarray(Wk, dtype=np.float32))
    bk = np.ascontiguousarray(np.asarray(bk, dtype=np.float32))

    if _NC_CACHE is None:
        _NC_CACHE = build_kernel()
    nc = _NC_CACHE

    in_maps = make_in_maps(query, target, mask, Wq, bq, Wk, bk)

    res = run_bass_kernel_spmd(nc, in_maps, list(range(NCORES)))
    outs = []
    for i in range(NCORES):
        aP = np.asarray(res.results[i]["alphaP"])  # [BS*CPB, 128]
        # undo the s = 1024h + 8p + j permutation
        a = aP.reshape(BS, 2, HK, 128).transpose(0, 1, 3, 2).reshape(BS, S)
        outs.append(a)
    return np.concatenate(outs, axis=0).astype(np.float32)


def make_in_maps(query, target, mask, Wq, bq, Wk, bk):
    # tiny derived tensors (B x E): q = query @ Wq.T + bq, qp = q @ Wk,
    # qb = q . bk — O(B*E*H) host prep vs the O(B*S*E) device stream
    q = query @ Wq.T + bq  # [B, H]
    qp_full = (q @ Wk).astype(np.float32)  # [B, E]
    qb_full = (q @ bk).astype(np.float32)  # [B]
    in_maps = []
    for i in range(NCORES):
        sl = slice(i * BS, (i + 1) * BS)
        m01 = (mask[sl] == 0).astype(np.float32)  # 1.0 keep / 0.0 masked
        m01P = np.ascontiguousarray(
            m01.reshape(BS, 2, 128, HK).transpose(2, 0, 1, 3).reshape(128, BS * CPB)
        )
        qbb = np.ascontiguousarray(
            np.broadcast_to(qb_full[sl][None, :], (128, BS)).astype(np.float32)
        )
        in_maps.append(
            {
                "target": target[sl],
                "pbb2": np.ascontiguousarray(
                    np.broadcast_to(
                        qp_full[sl][0:2].reshape(1, 2 * E), (128, 2 * E)
                    )
                ),
                "qpT": np.ascontiguousarray(
                    qp_full[sl].reshape(BS, 4, 128).transpose(2, 1, 0).reshape(128, 4 * BS)
                ),
                "qbb": qbb,
                "m01P": m01P,
            }
        )
    return in_maps
